# revision 11
# baseline (speedup 1.0000x reference)
"""Trainium2 Bass kernel for nn_Attention_45148696216391.

Multi-head attention with QK L2-norm (qk-norm) + learned per-head scales:
  q = x @ Wq.T ; k = x @ Wk.T ; v = x @ Wv.T       (per head, dh=64)
  q = l2norm(q) * q_scale ; k = l2norm(k) * k_scale
  out = softmax(q k^T / sqrt(dh)) @ v ; out = out @ Wo.T + bo

Sharding (8 cores): data parallel over batch b (2) x tensor parallel over
heads (16 heads -> 4 per core).  Each core computes, for its (b, head-group):
    P_out^T = Wo_s^T @ O^T   in (d, n) layout  -- a PARTIAL sum over e-dims.
Host reduces the 4 head-group partials per batch, transposes, adds bo.

Per-core layouts (all transposed, d/e on partitions):
  xt  (1024, 2048) = x[b].T
  Q^T/K^T chunks [128, 2048]: chunk ec rows = heads (2ec, 2ec+1) x dh(64)
  V natural in VA [128(j), 16(j-chunk), 4*65]: per head 64 V cols + ones col
    (ones col makes the PV matmul also produce the softmax denominator Z).
  scores S^T [j, i] per (chunk, i512) in psum pairs [128, 1024] (2 j-tiles)
  exp on ACT; PV accumulates O^T[dh(+Z), i] over 16 j-tiles.
  epilogue: r = 1/Z (row 64), replicate via K=1 ones-matmul, O^T *= r.
  out-proj: psum_po[d-tile, i512] = sum_ec WoT[ec] @ OCAT[ec] -> DMA to DRAM.

No max-subtraction in softmax: q,k are unit vectors so |s| <= |qs*ks|/8.
No eps clamp in l2norm: ||q|| ~ 5 for these inputs, eps=1e-12 unreachable.
q_scale/sqrt(dh) and k_scale are folded into the host-built replication masks.
"""

import os
import sys

sys.path.insert(0, "/opt/trn_rl_repo")

import numpy as np

import concourse.bacc as bacc
import concourse.mybir as mybir
import concourse.tile as tile

B, N, DIM = 2, 2048, 1024
H, DH = 16, 64
E = 256            # inner dims per core (4 heads x 64)
NC = 8             # cores
HPC = 4            # heads per core
I512 = 512         # attention i-tile
NI = N // I512     # 4 i-tiles
NDC = DIM // 128   # 8 d-chunks
NJT = N // 128     # 16 j-tiles

f32 = mybir.dt.float32
f32r = mybir.dt.float32r

# matmul operand dtype: "f32r" (full PE rate at N>=256) or "f32" (exact, 4x slower)
MM_DT = os.environ.get("KMM_DT", "f32r")


MMD = f32r if MM_DT == "f32r" else f32


def _r(ap):
    return ap


def build_nc():
    nc = bacc.Bacc("TRN2", target_bir_lowering=False, debug=False)

    xt = nc.dram_tensor("xt", [DIM, N], MMD, kind="ExternalInput").ap()
    wqt = nc.dram_tensor("wqt", [DIM, E], MMD, kind="ExternalInput").ap()
    wkt = nc.dram_tensor("wkt", [DIM, E], MMD, kind="ExternalInput").ap()
    wvt = nc.dram_tensor("wvt", [DIM, E], MMD, kind="ExternalInput").ap()
    wot = nc.dram_tensor("wot", [E, DIM], MMD, kind="ExternalInput").ap()
    hmk = nc.dram_tensor("hmk", [128, 66], MMD, kind="ExternalInput").ap()
    qmk = nc.dram_tensor("qmk", [2, 2, 128], MMD, kind="ExternalInput").ap()
    kmk = nc.dram_tensor("kmk", [2, 2, 128], MMD, kind="ExternalInput").ap()
    out = nc.dram_tensor("out", [DIM, N], f32, kind="ExternalOutput").ap()

    with tile.TileContext(nc) as tc:
        with (
            tc.tile_pool(name="wpool", bufs=1) as wpool,
            tc.tile_pool(name="big", bufs=1) as big,
            tc.tile_pool(name="xts", bufs=11) as xts,
            tc.tile_pool(name="sqp", bufs=3) as sqp,
            tc.tile_pool(name="nsp", bufs=4) as nsp,
            tc.tile_pool(name="ptp", bufs=4) as ptp,
            tc.tile_pool(name="obp", bufs=3) as obp,
            tc.tile_pool(name="pa", bufs=3, space="PSUM") as pa,
            tc.tile_pool(name="po", bufs=2, space="PSUM") as po,
        ):
            # ---- weights + constants in SBUF ----
            WQT = wpool.tile([128, NDC, E], MMD)  # [d_in_chunk, dc, e]
            WKT = wpool.tile([128, NDC, E], MMD)
            WVT = wpool.tile([128, NDC, E], MMD)
            WOT = wpool.tile([128, 2, DIM], MMD)  # [e_in_chunk, ec, d]
            nc.sync.dma_start(WQT[:], wqt.rearrange("(dc p) e -> p dc e", p=128))
            nc.sync.dma_start(WKT[:], wkt.rearrange("(dc p) e -> p dc e", p=128))
            nc.sync.dma_start(WVT[:], wvt.rearrange("(dc p) e -> p dc e", p=128))
            nc.sync.dma_start(WOT[:], wot.rearrange("(ec p) d -> p ec d", p=128))
            HM = wpool.tile([128, 66], MMD)  # cols 0-1: head mask; 2-65: ones
            nc.sync.dma_start(HM[:], hmk)
            QM = wpool.tile([2, 2, 128], MMD)
            KM = wpool.tile([2, 2, 128], MMD)
            nc.sync.dma_start(QM[:], qmk)
            nc.sync.dma_start(KM[:], kmk)


            # ---- big persistent tiles ----
            QT = [big.tile([128, N], MMD, name=f"qt{c}", tag=f"qt{c}") for c in range(2)]
            KT = [big.tile([128, N], MMD, name=f"kt{c}", tag=f"kt{c}") for c in range(2)]
            OC = [big.tile([128, N], MMD, name=f"oc{c}", tag=f"oc{c}") for c in range(2)]
            VA = big.tile([128, NJT, HPC * 65], MMD, name="va")
            for h in range(HPC):
                nc.vector.tensor_copy(
                    VA[:, :, 65 * h + 64 : 65 * h + 65],
                    HM[:, 2:3].to_broadcast([128, NJT, 1]),
                )

            # ---- projections: for each i512 block produce V, K, Q ----
            for i5 in range(NI):
                isl = slice(i5 * I512, (i5 + 1) * I512)
                xtl = []
                for dc in range(NDC):
                    t = xts.tile([128, I512], MMD, tag="xt")
                    nc.sync.dma_start(t[:], xt[128 * dc : 128 * (dc + 1), isl])
                    xtl.append(t)

                # V: natural layout, x^T tiles stationary
                for ntl in range(4):
                    nt = 4 * i5 + ntl
                    pv = pa.tile([128, E], f32, tag="A", name="pv")
                    for dc in range(NDC):
                        nc.tensor.matmul(
                            pv[:],
                            _r(xtl[dc][:, 128 * ntl : 128 * (ntl + 1)]),
                            _r(WVT[:, dc, :]),
                            start=(dc == 0),
                            stop=(dc == NDC - 1),
                        )
                    # scatter 4 heads' 64-col blocks into VA (ones cols untouched)
                    nc.vector.tensor_copy(
                        VA[:, nt, :].rearrange("p (h c) -> p h c", c=65)[:, :, 0:64],
                        pv[:].rearrange("p (h c) -> p h c", c=64),
                    )

                # K then Q: transposed layout, weights stationary + qk-norm
                for which, WT, MSK, DST in (
                    ("k", WKT, KM, KT),
                    ("q", WQT, QM, QT),
                ):
                    for ec in range(2):
                        pq = pa.tile([128, I512], f32, tag="A", name="pq")
                        for dc in range(NDC):
                            nc.tensor.matmul(
                                pq[:],
                                _r(WT[:, dc, 128 * ec : 128 * (ec + 1)]),
                                _r(xtl[dc][:]),
                                start=(dc == 0),
                                stop=(dc == NDC - 1),
                            )
                        # raw q/k to SBUF (releases the accumulator slot)
                        nc.vector.tensor_copy(DST[ec][:, isl], pq[:])
                        # sum of squares per (head, i) via mask matmul
                        sq = sqp.tile([128, I512], MMD, tag="sq")
                        nc.vector.tensor_tensor(
                            sq[:], DST[ec][:, isl], DST[ec][:, isl], mybir.AluOpType.mult
                        )
                        pnn = pa.tile([2, I512], f32, tag="A", name="pnn")
                        nc.tensor.matmul(
                            pnn[:], _r(HM[:, 0:2]), _r(sq[:]), start=True, stop=True
                        )
                        # ||q|| then 1/||q||
                        ns = nsp.tile([2, I512], f32, tag="ns")
                        nc.scalar.activation(
                            ns[:], pnn[:], mybir.ActivationFunctionType.Sqrt
                        )
                        rq = nsp.tile([2, I512], MMD, tag="rq")
                        with nc.allow_low_precision(reason="f32r storage, ~19-bit mantissa is plenty for 1/||q||"):
                            nc.vector.reciprocal(rq[:], ns[:])
                        # replicate to 128 rows with per-head scale folded in
                        prr = pa.tile([128, I512], f32, tag="A", name="prr")
                        nc.tensor.matmul(
                            prr[:], _r(MSK[:, ec, :]), _r(rq[:]), start=True, stop=True
                        )
                        nc.vector.tensor_tensor(
                            DST[ec][:, isl], DST[ec][:, isl], prr[:], mybir.AluOpType.mult
                        )

            # ---- attention ----
            for c in range(2):
                for i5 in range(NI):
                    isl = slice(i5 * I512, (i5 + 1) * I512)
                    pos = [po.tile([65, I512], f32, tag="po", name=f"po{_d}") for _d in range(2)]
                    for jp in range(NJT // 2):  # pairs of j-tiles
                        pts = []
                        for d in range(2):  # head within chunk (partition half)
                            rsl = slice(64 * d, 64 * (d + 1))
                            psc = pa.tile([128, 1024], f32, tag="A", name="psc")
                            for u in range(2):
                                jt = 2 * jp + u
                                nc.tensor.matmul(
                                    psc[:, 512 * u : 512 * (u + 1)],
                                    _r(KT[c][rsl, 128 * jt : 128 * (jt + 1)]),
                                    _r(QT[c][rsl, isl]),
                                    start=True,
                                    stop=True,
                                )
                            pt = ptp.tile([128, 1024], MMD, tag="pt")
                            nc.scalar.activation(
                                pt[:], psc[:], mybir.ActivationFunctionType.Exp
                            )
                            pts.append(pt)
                        for d in range(2):
                            h = 2 * c + d
                            for u in range(2):
                                jt = 2 * jp + u
                                nc.tensor.matmul(
                                    pos[d][:],
                                    _r(VA[:, jt, 65 * d + 65 * 2 * c : 65 * d + 65 * 2 * c + 65]),
                                    _r(pts[d][:, 512 * u : 512 * (u + 1)]),
                                    start=(jt == 0),
                                    stop=(jt == NJT - 1),
                                )
                    # epilogue: divide by Z (row 64), write into OCAT
                    for d in range(2):
                        rz = nsp.tile([65, I512], MMD, tag="rz")
                        with nc.allow_low_precision(reason="f32r storage for 1/Z"):
                            nc.vector.reciprocal(rz[64:65, :], pos[d][64:65, :])
                        prz = pa.tile([64, I512], f32, tag="A", name="prz")
                        nc.tensor.matmul(
                            prz[:],
                            _r(HM[64:65, 2:66]),
                            _r(rz[64:65, :]),
                            start=True,
                            stop=True,
                        )
                        rzr = nsp.tile([64, I512], MMD, tag="rzr")
                        nc.vector.tensor_copy(rzr[:], prz[:])
                        nc.vector.tensor_tensor(
                            OC[c][64 * d : 64 * (d + 1), isl],
                            pos[d][0:64, :],
                            rzr[:],
                            mybir.AluOpType.mult,
                        )

            # ---- output projection (partial over this core's e-slice) ----
            for i5 in range(NI):
                isl = slice(i5 * I512, (i5 + 1) * I512)
                for dt in range(NDC):
                    pp_o = pa.tile([128, I512], f32, tag="A", name="ppo")
                    for ec in range(2):
                        nc.tensor.matmul(
                            pp_o[:],
                            _r(WOT[:, ec, 128 * dt : 128 * (dt + 1)]),
                            _r(OC[ec][:, isl]),
                            start=(ec == 0),
                            stop=(ec == 1),
                        )
                    ob = obp.tile([128, I512], f32, tag="ob")
                    nc.vector.tensor_copy(ob[:], pp_o[:])
                    nc.sync.dma_start(out[128 * dt : 128 * (dt + 1), isl], ob[:])

    nc.compile()
    return nc


def make_in_maps(x, Wq, Wk, Wv, Wo, q_scale, k_scale):
    """Shard + lay out the full inputs for the 8 cores."""
    x = np.asarray(x, dtype=np.float32)
    Wq = np.asarray(Wq, dtype=np.float32)
    Wk = np.asarray(Wk, dtype=np.float32)
    Wv = np.asarray(Wv, dtype=np.float32)
    Wo = np.asarray(Wo, dtype=np.float32)
    qs = np.asarray(q_scale, dtype=np.float32).reshape(H, DH)
    ks = np.asarray(k_scale, dtype=np.float32).reshape(H, DH)

    hmk = np.zeros((128, 66), np.float32)
    hmk[0:64, 0] = 1.0
    hmk[64:128, 1] = 1.0
    hmk[:, 2:66] = 1.0

    xts = [np.ascontiguousarray(x[b].T) for b in range(B)]
    in_maps = []
    for core in range(NC):
        b, g = divmod(core, 4)
        esl = slice(E * g, E * (g + 1))
        qmk = np.zeros((2, 2, 128), np.float32)
        kmk = np.zeros((2, 2, 128), np.float32)
        for ec in range(2):
            for hh in range(2):
                head = HPC * g + 2 * ec + hh
                # lhsT layout for the replication matmul: [k(=hh) partition, ec, m]
                qmk[hh, ec, 64 * hh : 64 * hh + 64] = qs[head] * (DH ** -0.5)
                kmk[hh, ec, 64 * hh : 64 * hh + 64] = ks[head]
        in_maps.append(
            {
                "xt": xts[b],
                "wqt": np.ascontiguousarray(Wq[esl].T),
                "wkt": np.ascontiguousarray(Wk[esl].T),
                "wvt": np.ascontiguousarray(Wv[esl].T),
                "wot": np.ascontiguousarray(Wo[:, esl].T),
                "hmk": hmk,
                "qmk": qmk,
                "kmk": kmk,
            }
        )
    return in_maps


def gather_output(results, bo):
    """results: list of 8 dicts with 'out' (1024, 2048) partial^T arrays."""
    bo = np.asarray(bo, dtype=np.float32)
    out = np.empty((B, N, DIM), np.float32)
    for b in range(B):
        acc = results[4 * b]["out"].astype(np.float32)
        for g in range(1, 4):
            acc = acc + results[4 * b + g]["out"]
        out[b] = acc.T + bo
    return out


_NC_CACHE = {}


def kernel(x, Wq, Wk, Wv, Wo, bo, q_scale, k_scale):
    from concourse.bass_utils import run_bass_kernel_spmd

    key = MM_DT
    if key not in _NC_CACHE:
        _NC_CACHE[key] = build_nc()
    nc = _NC_CACHE[key]
    in_maps = make_in_maps(x, Wq, Wk, Wv, Wo, q_scale, k_scale)
    res = run_bass_kernel_spmd(nc, in_maps, list(range(NC)))
    return gather_output(res.results, bo)


# revision 14
# speedup vs baseline: 1.2689x; 1.2689x over previous
"""Trainium2 Bass kernel for nn_Attention_45148696216391.

Multi-head attention with QK L2-norm (qk-norm) + learned per-head scales:
  q = x @ Wq.T ; k = x @ Wk.T ; v = x @ Wv.T       (per head, dh=64)
  q = l2norm(q) * q_scale ; k = l2norm(k) * k_scale
  out = softmax(q k^T / sqrt(dh)) @ v ; out = out @ Wo.T + bo

Sharding (8 cores): data parallel over batch b (2) x tensor parallel over
heads (16 heads -> 4 per core).  Each core computes, for its (b, head-group):
    P_out^T = Wo_s^T @ O^T   in (d, n) layout  -- a PARTIAL sum over e-dims.
Host reduces the 4 head-group partials per batch, transposes, adds bo.

Per-core layouts (all transposed, d/e on partitions):
  xt  (1024, 2048) = x[b].T
  Q^T/K^T chunks [128, 2048]: chunk ec rows = heads (2ec, 2ec+1) x dh(64)
  V natural in VA [128(j), 16(j-chunk), 4*65]: per head 64 V cols + ones col
    (ones col makes the PV matmul also produce the softmax denominator Z).
  scores S^T [j, i] per (chunk, i512) in psum pairs [128, 1024] (2 j-tiles)
  exp on ACT; PV accumulates O^T[dh(+Z), i] over 16 j-tiles.
  epilogue: r = 1/Z (row 64), replicate via K=1 ones-matmul, O^T *= r.
  out-proj: psum_po[d-tile, i512] = sum_ec WoT[ec] @ OCAT[ec] -> DMA to DRAM.

No max-subtraction in softmax: q,k are unit vectors so |s| <= |qs*ks|/8.
No eps clamp in l2norm: ||q|| ~ 5 for these inputs, eps=1e-12 unreachable.
q_scale/sqrt(dh) and k_scale are folded into the host-built replication masks.
"""

import os
import sys

sys.path.insert(0, "/opt/trn_rl_repo")

import numpy as np

import concourse.bacc as bacc
import concourse.mybir as mybir
import concourse.tile as tile

B, N, DIM = 2, 2048, 1024
H, DH = 16, 64
E = 256            # inner dims per core (4 heads x 64)
NC = 8             # cores
HPC = 4            # heads per core
I512 = 512         # attention i-tile
NI = N // I512     # 4 i-tiles
NDC = DIM // 128   # 8 d-chunks
NJT = N // 128     # 16 j-tiles

f32 = mybir.dt.float32
f32r = mybir.dt.float32r
bf16 = mybir.dt.bfloat16

# matmul operand dtype: bf16 (full PE rate, FWL, HAM warms) | f32r | f32
MM_DT = os.environ.get("KMM_DT", "bf16")
MMD = {"bf16": bf16, "f32r": f32r, "f32": f32}[MM_DT]


def _r(ap):
    return ap


def build_nc():
    nc = bacc.Bacc("TRN2", target_bir_lowering=False, debug=False)

    xt = nc.dram_tensor("xt", [DIM, N], MMD, kind="ExternalInput").ap()
    wqt = nc.dram_tensor("wqt", [DIM, E], MMD, kind="ExternalInput").ap()
    wkt = nc.dram_tensor("wkt", [DIM, E], MMD, kind="ExternalInput").ap()
    wvt = nc.dram_tensor("wvt", [DIM, E], MMD, kind="ExternalInput").ap()
    wot = nc.dram_tensor("wot", [E, DIM], MMD, kind="ExternalInput").ap()
    hmk = nc.dram_tensor("hmk", [128, 66], MMD, kind="ExternalInput").ap()
    qmk = nc.dram_tensor("qmk", [2, 2, 128], MMD, kind="ExternalInput").ap()
    kmk = nc.dram_tensor("kmk", [2, 2, 128], MMD, kind="ExternalInput").ap()
    out = nc.dram_tensor("out", [DIM, N], f32, kind="ExternalOutput").ap()

    with tile.TileContext(nc) as tc:
        with (
            tc.tile_pool(name="wpool", bufs=1) as wpool,
            tc.tile_pool(name="big", bufs=1) as big,
            tc.tile_pool(name="xts", bufs=11) as xts,
            tc.tile_pool(name="sqp", bufs=3) as sqp,
            tc.tile_pool(name="nsp", bufs=4) as nsp,
            tc.tile_pool(name="ptp", bufs=4) as ptp,
            tc.tile_pool(name="obp", bufs=3) as obp,
            tc.tile_pool(name="zdp", bufs=4, space="DRAM") as zdp,
            tc.tile_pool(name="pa", bufs=3, space="PSUM") as pa,
            tc.tile_pool(name="po", bufs=2, space="PSUM") as po,
        ):
            # ---- weights + constants in SBUF ----
            WQT = wpool.tile([128, NDC, E], MMD)  # [d_in_chunk, dc, e]
            WKT = wpool.tile([128, NDC, E], MMD)
            WVT = wpool.tile([128, NDC, E], MMD)
            WOT = wpool.tile([128, 2, DIM], MMD)  # [e_in_chunk, ec, d]
            nc.sync.dma_start(WQT[:], wqt.rearrange("(dc p) e -> p dc e", p=128))
            nc.sync.dma_start(WKT[:], wkt.rearrange("(dc p) e -> p dc e", p=128))
            nc.sync.dma_start(WVT[:], wvt.rearrange("(dc p) e -> p dc e", p=128))
            nc.sync.dma_start(WOT[:], wot.rearrange("(ec p) d -> p ec d", p=128))
            HM = wpool.tile([128, 66], MMD)  # cols 0-1: head mask; 2-65: ones
            nc.sync.dma_start(HM[:], hmk)
            QM = wpool.tile([2, 2, 128], MMD)
            KM = wpool.tile([2, 2, 128], MMD)
            nc.sync.dma_start(QM[:], qmk)
            nc.sync.dma_start(KM[:], kmk)


            # ---- big persistent tiles ----
            QT = [big.tile([128, N], MMD, name=f"qt{c}", tag=f"qt{c}") for c in range(2)]
            KT = [big.tile([128, N], MMD, name=f"kt{c}", tag=f"kt{c}") for c in range(2)]
            OC = [big.tile([128, N], MMD, name=f"oc{c}", tag=f"oc{c}") for c in range(2)]
            VA = big.tile([128, NJT, HPC * 65], MMD, name="va")
            for h in range(HPC):
                nc.vector.tensor_copy(
                    VA[:, :, 65 * h + 64 : 65 * h + 65],
                    HM[:, 2:3].to_broadcast([128, NJT, 1]),
                )

            # ---- projections: for each i512 block produce V, K, Q ----
            for i5 in range(NI):
                isl = slice(i5 * I512, (i5 + 1) * I512)
                xtl = []
                for dc in range(NDC):
                    t = xts.tile([128, I512], MMD, tag="xt")
                    nc.sync.dma_start(t[:], xt[128 * dc : 128 * (dc + 1), isl])
                    xtl.append(t)

                # V: natural layout, x^T tiles stationary
                for ntl in range(4):
                    nt = 4 * i5 + ntl
                    pv = pa.tile([128, E], f32, tag="A", name="pv")
                    for dc in range(NDC):
                        nc.tensor.matmul(
                            pv[:],
                            _r(xtl[dc][:, 128 * ntl : 128 * (ntl + 1)]),
                            _r(WVT[:, dc, :]),
                            start=(dc == 0),
                            stop=(dc == NDC - 1),
                        )
                    # scatter 4 heads' 64-col blocks into VA (ones cols untouched)
                    nc.vector.tensor_copy(
                        VA[:, nt, :].rearrange("p (h c) -> p h c", c=65)[:, :, 0:64],
                        pv[:].rearrange("p (h c) -> p h c", c=64),
                    )

                # K then Q: transposed layout, weights stationary + qk-norm
                for which, WT, MSK, DST in (
                    ("k", WKT, KM, KT),
                    ("q", WQT, QM, QT),
                ):
                    for ec in range(2):
                        pq = pa.tile([128, I512], f32, tag="A", name="pq")
                        for dc in range(NDC):
                            nc.tensor.matmul(
                                pq[:],
                                _r(WT[:, dc, 128 * ec : 128 * (ec + 1)]),
                                _r(xtl[dc][:]),
                                start=(dc == 0),
                                stop=(dc == NDC - 1),
                            )
                        # raw q/k to SBUF (releases the accumulator slot)
                        nc.vector.tensor_copy(DST[ec][:, isl], pq[:])
                        # sum of squares per (head, i) via mask matmul
                        sq = sqp.tile([128, I512], MMD, tag="sq")
                        nc.vector.tensor_tensor(
                            sq[:], DST[ec][:, isl], DST[ec][:, isl], mybir.AluOpType.mult
                        )
                        pnn = pa.tile([2, I512], f32, tag="A", name="pnn")
                        nc.tensor.matmul(
                            pnn[:], _r(HM[:, 0:2]), _r(sq[:]), start=True, stop=True
                        )
                        # ||q|| then 1/||q||
                        ns = nsp.tile([2, I512], f32, tag="ns")
                        nc.scalar.activation(
                            ns[:], pnn[:], mybir.ActivationFunctionType.Sqrt
                        )
                        rq = nsp.tile([2, I512], MMD, tag="rq")
                        with nc.allow_low_precision(reason="per-head row scale; softmax is insensitive to 0.4% row-scale error"):
                            nc.vector.reciprocal(rq[:], ns[:])
                        # replicate to 128 rows with per-head scale folded in
                        prr = pa.tile([128, I512], f32, tag="A", name="prr")
                        nc.tensor.matmul(
                            prr[:], _r(MSK[:, ec, :]), _r(rq[:]), start=True, stop=True
                        )
                        nc.vector.tensor_tensor(
                            DST[ec][:, isl], DST[ec][:, isl], prr[:], mybir.AluOpType.mult
                        )

            # ---- attention ----
            for c in range(2):
                for i5 in range(NI):
                    isl = slice(i5 * I512, (i5 + 1) * I512)
                    pos = [po.tile([65, I512], f32, tag="po", name=f"po{_d}") for _d in range(2)]
                    for jp in range(NJT // 2):  # pairs of j-tiles
                        pscs = [
                            pa.tile([128, 1024], f32, tag="A", name=f"psc{_d}")
                            for _d in range(2)
                        ]
                        # interleave the two heads' score matmuls: they sit on
                        # different PE row groups (rows 0-63 vs 64-127) and run
                        # concurrently when adjacent in the stream
                        for u in range(2):
                            jt = 2 * jp + u
                            for d in range(2):
                                rsl = slice(64 * d, 64 * (d + 1))
                                nc.tensor.matmul(
                                    pscs[d][:, 512 * u : 512 * (u + 1)],
                                    _r(KT[c][rsl, 128 * jt : 128 * (jt + 1)]),
                                    _r(QT[c][rsl, isl]),
                                    start=True,
                                    stop=True,
                                )
                        pts = []
                        for d in range(2):
                            pt = ptp.tile([128, 1024], MMD, tag="pt")
                            nc.scalar.activation(
                                pt[:], pscs[d][:], mybir.ActivationFunctionType.Exp
                            )
                            pts.append(pt)
                        for d in range(2):
                            h = 2 * c + d
                            for u in range(2):
                                jt = 2 * jp + u
                                nc.tensor.matmul(
                                    pos[d][:],
                                    _r(VA[:, jt, 65 * d + 65 * 2 * c : 65 * d + 65 * 2 * c + 65]),
                                    _r(pts[d][:, 512 * u : 512 * (u + 1)]),
                                    start=(jt == 0),
                                    stop=(jt == NJT - 1),
                                )
                    # epilogue: divide by Z (row 64), write into OCAT.
                    # 1/Z kept in f32: bounce through DRAM to replicate it
                    # across 64 partitions (engines can't partition-broadcast).
                    for d in range(2):
                        rz = nsp.tile([65, I512], f32, tag="rz")
                        nc.vector.reciprocal(rz[64:65, :], pos[d][64:65, :])
                        zd = zdp.tile([1, I512], f32, tag="zd")
                        nc.sync.dma_start(zd[:], rz[64:65, :])
                        rzr = nsp.tile([64, I512], f32, tag="rzr")
                        nc.sync.dma_start(rzr[:], zd[:].to_broadcast([64, I512]))
                        nc.vector.tensor_tensor(
                            OC[c][64 * d : 64 * (d + 1), isl],
                            pos[d][0:64, :],
                            rzr[:],
                            mybir.AluOpType.mult,
                        )

            # ---- output projection (partial over this core's e-slice) ----
            for i5 in range(NI):
                isl = slice(i5 * I512, (i5 + 1) * I512)
                for dt in range(NDC):
                    pp_o = pa.tile([128, I512], f32, tag="A", name="ppo")
                    for ec in range(2):
                        nc.tensor.matmul(
                            pp_o[:],
                            _r(WOT[:, ec, 128 * dt : 128 * (dt + 1)]),
                            _r(OC[ec][:, isl]),
                            start=(ec == 0),
                            stop=(ec == 1),
                        )
                    ob = obp.tile([128, I512], f32, tag="ob")
                    nc.vector.tensor_copy(ob[:], pp_o[:])
                    nc.sync.dma_start(out[128 * dt : 128 * (dt + 1), isl], ob[:])

    nc.compile()
    return nc


def make_in_maps(x, Wq, Wk, Wv, Wo, q_scale, k_scale):
    """Shard + lay out the full inputs for the 8 cores."""
    npdt = mybir.dt.np(MMD)
    x = np.asarray(x, dtype=np.float32)
    Wq = np.asarray(Wq, dtype=np.float32)
    Wk = np.asarray(Wk, dtype=np.float32)
    Wv = np.asarray(Wv, dtype=np.float32)
    Wo = np.asarray(Wo, dtype=np.float32)
    qs = np.asarray(q_scale, dtype=np.float32).reshape(H, DH)
    ks = np.asarray(k_scale, dtype=np.float32).reshape(H, DH)

    hmk = np.zeros((128, 66), np.float32)
    hmk[0:64, 0] = 1.0
    hmk[64:128, 1] = 1.0
    hmk[:, 2:66] = 1.0

    xts = [np.ascontiguousarray(x[b].T).astype(npdt) for b in range(B)]
    hmk = hmk.astype(npdt)
    in_maps = []
    for core in range(NC):
        b, g = divmod(core, 4)
        esl = slice(E * g, E * (g + 1))
        qmk = np.zeros((2, 2, 128), np.float32)
        kmk = np.zeros((2, 2, 128), np.float32)
        for ec in range(2):
            for hh in range(2):
                head = HPC * g + 2 * ec + hh
                # lhsT layout for the replication matmul: [k(=hh) partition, ec, m]
                qmk[hh, ec, 64 * hh : 64 * hh + 64] = qs[head] * (DH ** -0.5)
                kmk[hh, ec, 64 * hh : 64 * hh + 64] = ks[head]
        in_maps.append(
            {
                "xt": xts[b],
                "wqt": np.ascontiguousarray(Wq[esl].T).astype(npdt),
                "wkt": np.ascontiguousarray(Wk[esl].T).astype(npdt),
                "wvt": np.ascontiguousarray(Wv[esl].T).astype(npdt),
                "wot": np.ascontiguousarray(Wo[:, esl].T).astype(npdt),
                "hmk": hmk,
                "qmk": qmk.astype(npdt),
                "kmk": kmk.astype(npdt),
            }
        )
    return in_maps


def gather_output(results, bo):
    """results: list of 8 dicts with 'out' (1024, 2048) partial^T arrays."""
    bo = np.asarray(bo, dtype=np.float32)
    out = np.empty((B, N, DIM), np.float32)
    for b in range(B):
        acc = results[4 * b]["out"].astype(np.float32)
        for g in range(1, 4):
            acc = acc + results[4 * b + g]["out"]
        out[b] = acc.T + bo
    return out


_NC_CACHE = {}


def kernel(x, Wq, Wk, Wv, Wo, bo, q_scale, k_scale):
    from concourse.bass_utils import run_bass_kernel_spmd

    key = MM_DT
    if key not in _NC_CACHE:
        _NC_CACHE[key] = build_nc()
    nc = _NC_CACHE[key]
    in_maps = make_in_maps(x, Wq, Wk, Wv, Wo, q_scale, k_scale)
    res = run_bass_kernel_spmd(nc, in_maps, list(range(NC)))
    return gather_output(res.results, bo)


# revision 17
# speedup vs baseline: 1.3912x; 1.0963x over previous
"""Trainium2 Bass kernel for nn_Attention_45148696216391.

Multi-head attention with QK L2-norm (qk-norm) + learned per-head scales:
  q = x @ Wq.T ; k = x @ Wk.T ; v = x @ Wv.T       (per head, dh=64)
  q = l2norm(q) * q_scale ; k = l2norm(k) * k_scale
  out = softmax(q k^T / sqrt(dh)) @ v ; out = out @ Wo.T + bo

Sharding (8 cores): data parallel over batch b (2) x tensor parallel over
heads (16 heads -> 4 per core).  Each core computes, for its (b, head-group):
    P_out^T = Wo_s^T @ O^T   in (d, n) layout  -- a PARTIAL sum over e-dims.
Host reduces the 4 head-group partials per batch, transposes, adds bo.

Per-core layouts (all transposed, d/e on partitions):
  xt  (1024, 2048) = x[b].T
  Q^T/K^T chunks [128, 2048]: chunk ec rows = heads (2ec, 2ec+1) x dh(64)
  V natural in VA [128(j), 16(j-chunk), 4*65]: per head 64 V cols + ones col
    (ones col makes the PV matmul also produce the softmax denominator Z).
  scores S^T [j, i] per (chunk, i512) in psum pairs [128, 1024] (2 j-tiles)
  exp on ACT; PV accumulates O^T[dh(+Z), i] over 16 j-tiles.
  epilogue: r = 1/Z (row 64), replicate via K=1 ones-matmul, O^T *= r.
  out-proj: psum_po[d-tile, i512] = sum_ec WoT[ec] @ OCAT[ec] -> DMA to DRAM.

No max-subtraction in softmax: q,k are unit vectors so |s| <= |qs*ks|/8.
No eps clamp in l2norm: ||q|| ~ 5 for these inputs, eps=1e-12 unreachable.
q_scale/sqrt(dh) and k_scale are folded into the host-built replication masks.
"""

import os
import sys

sys.path.insert(0, "/opt/trn_rl_repo")

import numpy as np

import concourse.bacc as bacc
import concourse.mybir as mybir
import concourse.tile as tile

B, N, DIM = 2, 2048, 1024
H, DH = 16, 64
E = 256            # inner dims per core (4 heads x 64)
NC = 8             # cores
HPC = 4            # heads per core
I512 = 512         # attention i-tile
NI = N // I512     # 4 i-tiles
NDC = DIM // 128   # 8 d-chunks
NJT = N // 128     # 16 j-tiles

f32 = mybir.dt.float32
f32r = mybir.dt.float32r
bf16 = mybir.dt.bfloat16

# matmul operand dtype: bf16 (full PE rate, FWL, HAM warms) | f32r | f32
MM_DT = os.environ.get("KMM_DT", "bf16")
MMD = {"bf16": bf16, "f32r": f32r, "f32": f32}[MM_DT]


def _r(ap):
    return ap


def build_nc():
    nc = bacc.Bacc("TRN2", target_bir_lowering=False, debug=False)

    xt = nc.dram_tensor("xt", [DIM, N], MMD, kind="ExternalInput").ap()
    wqt = nc.dram_tensor("wqt", [DIM, E], MMD, kind="ExternalInput").ap()
    wkt = nc.dram_tensor("wkt", [DIM, E], MMD, kind="ExternalInput").ap()
    wvt = nc.dram_tensor("wvt", [DIM, E], MMD, kind="ExternalInput").ap()
    wot = nc.dram_tensor("wot", [E, DIM], MMD, kind="ExternalInput").ap()
    hmk = nc.dram_tensor("hmk", [128, 66], MMD, kind="ExternalInput").ap()
    qsc = nc.dram_tensor("qsc", [128, 2], f32, kind="ExternalInput").ap()
    ksc = nc.dram_tensor("ksc", [128, 2], f32, kind="ExternalInput").ap()
    out = nc.dram_tensor("out", [DIM, N], f32, kind="ExternalOutput").ap()

    with tile.TileContext(nc) as tc:
        with (
            tc.tile_pool(name="wpool", bufs=1) as wpool,
            tc.tile_pool(name="big", bufs=1) as big,
            tc.tile_pool(name="xts", bufs=11) as xts,
            tc.tile_pool(name="sqp", bufs=3) as sqp,
            tc.tile_pool(name="nsp", bufs=4) as nsp,
            tc.tile_pool(name="ptp", bufs=4) as ptp,
            tc.tile_pool(name="obp", bufs=3) as obp,
            tc.tile_pool(name="zdp", bufs=4, space="DRAM") as zdp,
            tc.tile_pool(name="pa", bufs=3, space="PSUM") as pa,
            tc.tile_pool(name="po", bufs=2, space="PSUM") as po,
        ):
            # ---- weights + constants in SBUF ----
            WQT = wpool.tile([128, NDC, E], MMD)  # [d_in_chunk, dc, e]
            WKT = wpool.tile([128, NDC, E], MMD)
            WVT = wpool.tile([128, NDC, E], MMD)
            WOT = wpool.tile([128, 2, DIM], MMD)  # [e_in_chunk, ec, d]
            nc.sync.dma_start(WQT[:], wqt.rearrange("(dc p) e -> p dc e", p=128))
            nc.sync.dma_start(WKT[:], wkt.rearrange("(dc p) e -> p dc e", p=128))
            nc.sync.dma_start(WVT[:], wvt.rearrange("(dc p) e -> p dc e", p=128))
            nc.sync.dma_start(WOT[:], wot.rearrange("(ec p) d -> p ec d", p=128))
            HM = wpool.tile([128, 66], MMD)  # cols 0-1: head mask; 2-65: ones
            nc.sync.dma_start(HM[:], hmk)
            QS = wpool.tile([128, 2], f32)
            KS = wpool.tile([128, 2], f32)
            nc.sync.dma_start(QS[:], qsc)
            nc.sync.dma_start(KS[:], ksc)


            # ---- big persistent tiles ----
            QT = [big.tile([128, N], MMD, name=f"qt{c}", tag=f"qt{c}") for c in range(2)]
            KT = [big.tile([128, N], MMD, name=f"kt{c}", tag=f"kt{c}") for c in range(2)]
            OC = [big.tile([128, N], MMD, name=f"oc{c}", tag=f"oc{c}") for c in range(2)]
            VA = big.tile([128, NJT, HPC * 65], MMD, name="va")
            for h in range(HPC):
                nc.vector.tensor_copy(
                    VA[:, :, 65 * h + 64 : 65 * h + 65],
                    HM[:, 2:3].to_broadcast([128, NJT, 1]),
                )

            # ---- projections: for each i512 block produce V, K, Q ----
            for i5 in range(NI):
                isl = slice(i5 * I512, (i5 + 1) * I512)
                xtl = []
                for dc in range(NDC):
                    t = xts.tile([128, I512], MMD, tag="xt")
                    nc.sync.dma_start(t[:], xt[128 * dc : 128 * (dc + 1), isl])
                    xtl.append(t)

                # K then Q: transposed layout, weights stationary + qk-norm
                for which, WT, SC, DST in (
                    ("k", WKT, KS, KT),
                    ("q", WQT, QS, QT),
                ):
                    for ec in range(2):
                        pq = pa.tile([128, I512], f32, tag="A", name="pq")
                        for dc in range(NDC):
                            nc.tensor.matmul(
                                pq[:],
                                _r(WT[:, dc, 128 * ec : 128 * (ec + 1)]),
                                _r(xtl[dc][:]),
                                start=(dc == 0),
                                stop=(dc == NDC - 1),
                            )
                        # sum of squares per (head, i): square on ACT (idle in
                        # this phase; Square shares the exp table set), reduce
                        # over the 64 dh partitions via mask matmul
                        sq = sqp.tile([128, I512], MMD, tag="sq")
                        nc.scalar.activation(
                            sq[:], pq[:], mybir.ActivationFunctionType.Square
                        )
                        pnn = pa.tile([2, I512], f32, tag="A", name="pnn")
                        nc.tensor.matmul(
                            pnn[:], _r(HM[:, 0:2]), _r(sq[:]), start=True, stop=True
                        )
                        # r = 1/||q|| in f32, replicated to the 128 partitions
                        # by a bounce through DRAM, with q_scale folded in
                        ns = nsp.tile([2, I512], f32, tag="ns")
                        nc.scalar.activation(
                            ns[:], pnn[:], mybir.ActivationFunctionType.Sqrt
                        )
                        rq = nsp.tile([2, I512], f32, tag="rq")
                        nc.vector.reciprocal(rq[:], ns[:])
                        rd = zdp.tile([2, I512], f32, tag="rd")
                        nc.sync.dma_start(rd[:], rq[:])
                        rr = sqp.tile([128, I512], f32, tag="rr")
                        nc.sync.dma_start(
                            rr[0:64, :], rd[0:1, :].to_broadcast([64, I512])
                        )
                        nc.sync.dma_start(
                            rr[64:128, :], rd[1:2, :].to_broadcast([64, I512])
                        )
                        nc.vector.tensor_scalar_mul(rr[:], rr[:], SC[:, ec : ec + 1])
                        nc.vector.tensor_tensor(
                            DST[ec][:, isl], pq[:], rr[:], mybir.AluOpType.mult
                        )

                # V: natural layout, x^T tiles stationary (emitted after K/Q so
                # its matmuls fill PE gaps during the norm epilogue chains)
                for ntl in range(4):
                    nt = 4 * i5 + ntl
                    pv = pa.tile([128, E], f32, tag="A", name="pv")
                    for dc in range(NDC):
                        nc.tensor.matmul(
                            pv[:],
                            _r(xtl[dc][:, 128 * ntl : 128 * (ntl + 1)]),
                            _r(WVT[:, dc, :]),
                            start=(dc == 0),
                            stop=(dc == NDC - 1),
                        )
                    # scatter 4 heads' 64-col blocks into VA (ones cols untouched)
                    nc.vector.tensor_copy(
                        VA[:, nt, :].rearrange("p (h c) -> p h c", c=65)[:, :, 0:64],
                        pv[:].rearrange("p (h c) -> p h c", c=64),
                    )

            # ---- attention ----
            for c in range(2):
                for i5 in range(NI):
                    isl = slice(i5 * I512, (i5 + 1) * I512)
                    pos = [po.tile([65, I512], f32, tag="po", name=f"po{_d}") for _d in range(2)]
                    for jp in range(NJT // 2):  # pairs of j-tiles
                        pscs = [
                            pa.tile([128, 1024], f32, tag="A", name=f"psc{_d}")
                            for _d in range(2)
                        ]
                        # interleave the two heads' score matmuls: they sit on
                        # different PE row groups (rows 0-63 vs 64-127) and run
                        # concurrently when adjacent in the stream
                        for u in range(2):
                            jt = 2 * jp + u
                            for d in range(2):
                                rsl = slice(64 * d, 64 * (d + 1))
                                nc.tensor.matmul(
                                    pscs[d][:, 512 * u : 512 * (u + 1)],
                                    _r(KT[c][rsl, 128 * jt : 128 * (jt + 1)]),
                                    _r(QT[c][rsl, isl]),
                                    start=True,
                                    stop=True,
                                )
                        pts = []
                        for d in range(2):
                            pt = ptp.tile([128, 1024], MMD, tag="pt")
                            nc.scalar.activation(
                                pt[:], pscs[d][:], mybir.ActivationFunctionType.Exp
                            )
                            pts.append(pt)
                        for d in range(2):
                            h = 2 * c + d
                            for u in range(2):
                                jt = 2 * jp + u
                                nc.tensor.matmul(
                                    pos[d][:],
                                    _r(VA[:, jt, 65 * d + 65 * 2 * c : 65 * d + 65 * 2 * c + 65]),
                                    _r(pts[d][:, 512 * u : 512 * (u + 1)]),
                                    start=(jt == 0),
                                    stop=(jt == NJT - 1),
                                )
                    # epilogue: divide by Z (row 64), write into OCAT.
                    # 1/Z kept in f32: bounce through DRAM to replicate it
                    # across 64 partitions (engines can't partition-broadcast).
                    for d in range(2):
                        rz = nsp.tile([65, I512], f32, tag="rz")
                        nc.vector.reciprocal(rz[64:65, :], pos[d][64:65, :])
                        zd = zdp.tile([1, I512], f32, tag="zd")
                        nc.sync.dma_start(zd[:], rz[64:65, :])
                        rzr = nsp.tile([64, I512], f32, tag="rzr")
                        nc.sync.dma_start(rzr[:], zd[:].to_broadcast([64, I512]))
                        nc.vector.tensor_tensor(
                            OC[c][64 * d : 64 * (d + 1), isl],
                            pos[d][0:64, :],
                            rzr[:],
                            mybir.AluOpType.mult,
                        )

            # ---- output projection (partial over this core's e-slice) ----
            for i5 in range(NI):
                isl = slice(i5 * I512, (i5 + 1) * I512)
                for dt in range(NDC):
                    pp_o = pa.tile([128, I512], f32, tag="A", name="ppo")
                    for ec in range(2):
                        nc.tensor.matmul(
                            pp_o[:],
                            _r(WOT[:, ec, 128 * dt : 128 * (dt + 1)]),
                            _r(OC[ec][:, isl]),
                            start=(ec == 0),
                            stop=(ec == 1),
                        )
                    ob = obp.tile([128, I512], f32, tag="ob")
                    nc.scalar.copy(ob[:], pp_o[:])
                    nc.sync.dma_start(out[128 * dt : 128 * (dt + 1), isl], ob[:])

    nc.compile()
    return nc


def make_in_maps(x, Wq, Wk, Wv, Wo, q_scale, k_scale):
    """Shard + lay out the full inputs for the 8 cores."""
    npdt = mybir.dt.np(MMD)
    x = np.asarray(x, dtype=np.float32)
    Wq = np.asarray(Wq, dtype=np.float32)
    Wk = np.asarray(Wk, dtype=np.float32)
    Wv = np.asarray(Wv, dtype=np.float32)
    Wo = np.asarray(Wo, dtype=np.float32)
    qs = np.asarray(q_scale, dtype=np.float32).reshape(H, DH)
    ks = np.asarray(k_scale, dtype=np.float32).reshape(H, DH)

    hmk = np.zeros((128, 66), np.float32)
    hmk[0:64, 0] = 1.0
    hmk[64:128, 1] = 1.0
    hmk[:, 2:66] = 1.0

    xts = [np.ascontiguousarray(x[b].T).astype(npdt) for b in range(B)]
    hmk = hmk.astype(npdt)
    in_maps = []
    for core in range(NC):
        b, g = divmod(core, 4)
        esl = slice(E * g, E * (g + 1))
        qsc = np.zeros((128, 2), np.float32)
        ksc = np.zeros((128, 2), np.float32)
        for ec in range(2):
            for hh in range(2):
                head = HPC * g + 2 * ec + hh
                qsc[64 * hh : 64 * hh + 64, ec] = qs[head] * (DH ** -0.5)
                ksc[64 * hh : 64 * hh + 64, ec] = ks[head]
        in_maps.append(
            {
                "xt": xts[b],
                "wqt": np.ascontiguousarray(Wq[esl].T).astype(npdt),
                "wkt": np.ascontiguousarray(Wk[esl].T).astype(npdt),
                "wvt": np.ascontiguousarray(Wv[esl].T).astype(npdt),
                "wot": np.ascontiguousarray(Wo[:, esl].T).astype(npdt),
                "hmk": hmk,
                "qsc": qsc,
                "ksc": ksc,
            }
        )
    return in_maps


def gather_output(results, bo):
    """results: list of 8 dicts with 'out' (1024, 2048) partial^T arrays."""
    bo = np.asarray(bo, dtype=np.float32)
    out = np.empty((B, N, DIM), np.float32)
    for b in range(B):
        acc = results[4 * b]["out"].astype(np.float32)
        for g in range(1, 4):
            acc = acc + results[4 * b + g]["out"]
        out[b] = acc.T + bo
    return out


_NC_CACHE = {}


def kernel(x, Wq, Wk, Wv, Wo, bo, q_scale, k_scale):
    from concourse.bass_utils import run_bass_kernel_spmd

    key = MM_DT
    if key not in _NC_CACHE:
        _NC_CACHE[key] = build_nc()
    nc = _NC_CACHE[key]
    in_maps = make_in_maps(x, Wq, Wk, Wv, Wo, q_scale, k_scale)
    res = run_bass_kernel_spmd(nc, in_maps, list(range(NC)))
    return gather_output(res.results, bo)
